# revision 13
# baseline (speedup 1.0000x reference)
"""Diag-embed kernel for Trainium2 (raw Bass, manual semaphores).

Problem: x [8192, 176] f32 -> out [8192, 176, 176] f32 with
out[i] = diag(x[i]).  Data-parallel over 8 NeuronCores: core c handles
batch rows [1024c, 1024(c+1)).

Per core the output block is 1024*176*176*4 B ~= 127 MB of mostly zeros
-> purely HBM-write bound (~358 GB/s/core => ~354 us floor).  Two
persistent SBUF templates tA/tB [128, 15488] f32 hold the first/second
half of 128 flattened diag matrices (partition p = chunk item p).  The
zero background is memset once; per chunk of 128 items only the 88
diagonal slots per half are refreshed with a strided DVE copy, then each
half streams out as one 7.9 MB contiguous DMA.  The A/B split lets chunk
n+1's scatter overlap chunk n's second-half DMA so the store queue never
idles.  Manual semaphores keep every instruction at <=1 sync wait (the
TRN2 codegen rejects more).
"""

import numpy as np

B_FULL = 8192
D = 176
DD = D * D            # 30976 floats per item
HALF = DD // 2        # 15488
N_CORES = 8
B_SHARD = B_FULL // N_CORES   # 1024
P = 128
N_CHUNKS = B_SHARD // P       # 8

# diag position j sits at flat offset j*(D+1); j in [0, 88) lands in the
# first half, j in [88, 176) in the second half at offset 88 + (j-88)*177.
ND = 88
LAST_A = (ND - 1) * (D + 1)   # 15399

_prog_cache = {}


def _build_program(repeat: int = 1, timing: bool = False):
    """repeat>1 re-runs the whole store pipeline (same output region)
    inside one NEFF.  timing=True redirects the big output to an internal
    DRAM scratch tensor (same HBM-write work) and exposes only a tiny
    [1,1] ExternalOutput, so benchmarking doesn't ship 1 GB over the
    axon relay.  Both knobs are for test.py only."""
    from concourse import bass, mybir

    f32 = mybir.dt.float32
    nc = bass.Bass(target_bir_lowering=False)

    x = nc.dram_tensor("x", [B_SHARD, D], f32, kind="ExternalInput")
    if timing:
        out = nc.dram_tensor("outscratch", [B_SHARD, D, D], f32)
        dummy = nc.dram_tensor("tiny_out", [P, 1], f32, kind="ExternalOutput")
    else:
        out = nc.dram_tensor("out", [B_SHARD, D, D], f32, kind="ExternalOutput")
        dummy = None
    out2d = out[:].rearrange("b i j -> b (i j)")   # [1024, 30976]
    # chunk n, partition p must read x[n*128 + p, :]
    x3d = x[:].rearrange("(n p) c -> p n c", p=P)  # [128, 8, 176]

    with (
        nc.semaphore("sem_x") as sem_x,
        nc.semaphore("sem_t") as sem_t,
        nc.semaphore("sem_a") as sem_a,
        nc.semaphore("sem_b") as sem_b,
        nc.semaphore("sem_sa") as sem_sa,
        nc.semaphore("sem_sb") as sem_sb,
        nc.sbuf_tensor("tA", [P, HALF], f32) as tA,
        nc.sbuf_tensor("tB", [P, HALF], f32) as tB,
        nc.sbuf_tensor("xall", [P, N_CHUNKS, D], f32) as xall,
    ):
        with nc.Block() as block:

            @block.scalar
            def _(act):
                # one strided HWDGE load brings in all of x, chunk-major
                act.dma_start(out=xall[:], in_=x3d).then_inc(sem_x, 16)

            @block.vector
            def _(v):
                v.memset(tA[:], 0.0)
                v.memset(tB[:], 0.0)
                for m in range(N_CHUNKS * repeat):
                    n = m % N_CHUNKS
                    iA = v.tensor_copy(
                        tA[:, 0 : LAST_A + 1 : D + 1], xall[:, n, 0:ND]
                    )
                    if m == 0:
                        iA.wait_op(sem_x, 16, "sem-ge")      # x resident
                    else:
                        iA.wait_op(sem_a, 16 * m, "sem-ge")  # WAR: dmaA(m-1)
                    iA.then_inc(sem_sa)

                    iB = v.tensor_copy(
                        tB[:, ND : HALF : D + 1], xall[:, n, ND:D]
                    )
                    if m > 0:
                        iB.wait_op(sem_b, 16 * m, "sem-ge")  # WAR: dmaB(m-1)
                    iB.then_inc(sem_sb)

            @block.sync
            def _(sp):
                for m in range(N_CHUNKS * repeat):
                    n = m % N_CHUNKS
                    rows = slice(n * P, (n + 1) * P)
                    dA = sp.dma_start(out=out2d[rows, 0:HALF], in_=tA[:])
                    dA.wait_op(sem_sa, m + 1, "sem-ge")      # RAW: scatterA(m)
                    dA.then_inc(sem_a, 16)
                    dB = sp.dma_start(out=out2d[rows, HALF:DD], in_=tB[:])
                    dB.wait_op(sem_sb, m + 1, "sem-ge")      # RAW: scatterB(m)
                    dB.then_inc(sem_b, 16)
                # all stores landed before the end-of-kernel barrier
                sp.wait_ge(sem_a, 16 * N_CHUNKS * repeat)
                sp.wait_ge(sem_b, 16 * N_CHUNKS * repeat)
                if dummy is not None:
                    d = sp.dma_start(out=dummy[:], in_=tA[:, 0:1])
                    d.then_inc(sem_t, 16)
                    sp.wait_ge(sem_t, 16)

    return nc


def _get_program(repeat: int = 1, timing: bool = False):
    key = ("nc", repeat, timing)
    if key not in _prog_cache:
        _prog_cache[key] = _build_program(repeat, timing)
    return _prog_cache[key]


def _run(x: np.ndarray, **spmd_kwargs):
    from concourse.bass_utils import run_bass_kernel_spmd

    x = np.ascontiguousarray(x, dtype=np.float32)
    assert x.shape == (B_FULL, D), x.shape
    nc = _get_program()
    in_maps = [
        {"x": x[c * B_SHARD : (c + 1) * B_SHARD]} for c in range(N_CORES)
    ]
    res = run_bass_kernel_spmd(nc, in_maps, list(range(N_CORES)), **spmd_kwargs)
    full = np.concatenate([r["out"] for r in res.results], axis=0)
    return full, res


def kernel(**inputs) -> np.ndarray:
    full, _ = _run(inputs["x"])
    return full
